# revision 21
# baseline (speedup 1.0000x reference)
"""DigitCaps (capsule routing) Trainium2 Bass kernel, v3.

u [512, 1152, 8] f32, W [1, 1152, 10, 16, 8] f32 -> v [512, 10, 16] f32
(3 dynamic-routing iterations, softmax over 10 classes).

Pure data-parallel: batch 64 per core x 8 cores; everything on-chip;
u_hat (377MB) never materialized. Per routing iteration:
  T[b,i,c,k] = sum_d W[i,c,d,k] v[b,c,d]     PE -> PSUM
  evac to bf16 (ACT) or fused mul (DVE-from-PSUM), P = T*u
  Linc[b,i,c] = sum_k P                      PE eye-matmul accumulate
  cE = exp(Linc) [* cE_prev]                 ACT exp from PSUM (+mul)
  den folds; recip DVE
  xc_c = (u*recT) * cE_c                     DVE / GPSIMD split
  s[b,c,:] = sum_{ik} W xc_c                 PE accumulating matmuls
  v = squash(s)
exp(L1+L2) == exp(L1)*exp(L2), so logits are never materialized.

v3 structure:
  - s0 accumulates per-k-chunk as the input DMA streams in; WT loads in
    pass-row chunks so round-1 T matmuls start as soon as possible.
  - vT staging via PE-transpose + DVE copies (no DRAM round trip).
  - eye (k-sum) matmuls of pass p-1 interleave into pass p's T window;
    exp fires mid-iteration so its PSUM tiles recycle without stalls.
  - s-phase of round 1 is interleaved with round 2's L-phase.
  - squash uses sum-of-squares on DVE (ttr) and sqrt via exp(0.5*ln x),
    keeping every ACT func in one table (no act-table reloads).

Layouts (per core, B=64):
  i: block g = i//128 (9 blocks), partition r = i%128
  class c = 2p+ch, pass p in [0,5), parity ch in {0,1}
  exp/cE: [r, p, ch, g, b]
"""

import os
import numpy as np

N_CORES = 8
B_PER = 64
I_CAPS = 1152
K_DIM = 8
C_CLS = 10
D_DIM = 16
NG = I_CAPS // 128  # 9
EPS = 1e-8


def _ktup(env, default):
    return tuple(
        int(x) for x in os.environ.get(env, default).split(",") if x != ""
    )


# --- schedule knobs (cost-model balancing) ---
Z_KS = _ktup("KV2_ZKS", "1,3,5,7")  # fused PSUM-mul k's (DVE, no evac)
Z4_KS = _ktup("KV2_Z4", os.environ.get("KV2_ZKS", "1,3,5,7"))  # pass-4 set
POOL_MUL_KS = _ktup("KV2_PMKS", "")  # evac'd k's whose mul runs on GPSIMD
XC_POOL = _ktup("KV2_XCPOOL", "")   # classes with whole xc mul on GPSIMD
XC_DMA0 = _ktup("KV2_XCDMA0", "")   # round-1 classes on the DMA-mult route
XC_DMA1 = _ktup("KV2_XCDMA1", "")   # round-2 classes on the DMA-mult route
XC_GPOOL = int(os.environ.get("KV2_XCGPOOL", "2"))   # merged-round g's on Pool
XC_GPOOL1 = int(os.environ.get("KV2_XCG1", "2"))     # final-round g's on Pool
FOLDS_POOL = os.environ.get("KV2_FOLDSPOOL", "1") == "1"
CE_POOL = os.environ.get("KV2_CEPOOL", "1") == "1"
FP8_T = os.environ.get("KV2_FP8", "0") == "1"  # DoubleRow fp8 T matmuls
W8SCALE = 256.0  # exact power-of-two prescale lifting fp8 W out of subnormals
WU_C = int(os.environ.get("KV2_WUC", "3"))   # warmup matmuls per s0 chunk
WU_T = int(os.environ.get("KV2_WUT", "6"))   # warmup before L round 1

_CACHE = {}

# eye(p-1) emission inside iteration p: k-step -> which eye k's to emit
EYE_SCHED = {0: (0, 1, 2), 1: (3, 4, 5), 2: (6, 7)}
# pass-4's own eye partials emitted late in its own k-loop
EYE4_SCHED = {5: (0, 1, 2, 3), 6: (4, 5)}
BLK = ((0, 384), (384, 768), (768, 1152))  # eye/exp blocks (3 g's each)


def _build():
    import concourse.bass as bass
    import concourse.mybir as mybir
    from concourse import tile, bacc

    f32 = mybir.dt.float32
    bf16 = mybir.dt.bfloat16
    f8 = mybir.dt.float8e4
    AF = mybir.ActivationFunctionType
    OP = mybir.AluOpType
    PM = mybir.MatmulPerfMode

    nc = bacc.Bacc()
    uTk_in = nc.dram_tensor(
        "uTk_h", [128, K_DIM, NG, B_PER], bf16, kind="ExternalInput"
    )
    wsk_in = nc.dram_tensor(
        "wsk_h", [128, K_DIM, NG, C_CLS, D_DIM], bf16, kind="ExternalInput"
    )
    # wt cols 0:1152 = rows 16c+d classes 0-7 (all 128 partitions);
    # cols 1152:2304 = rows 16(c-8)+d classes 8,9 (partitions 0:32)
    if FP8_T:
        # DoubleRow layout: [32p+8cc+d//2, k, d%2, i(+1152 for classes 8,9)]
        wt_in = nc.dram_tensor(
            "wt_h", [128, K_DIM, 2, 2 * I_CAPS], f8, kind="ExternalInput"
        )
    else:
        wt_in = nc.dram_tensor(
            "wt_h", [128, K_DIM, 2 * I_CAPS], bf16, kind="ExternalInput"
        )
    eyebf = nc.dram_tensor("eyebf", [128, 128], bf16, kind="ExternalInput")
    eyef32 = nc.dram_tensor("eyef32", [64, 64], f32, kind="ExternalInput")
    v_out = nc.dram_tensor("v", [B_PER, C_CLS, D_DIM], f32, kind="ExternalOutput")

    with tile.TileContext(nc) as tc:
        perm = tc.alloc_tile_pool(name="perm", bufs=1)
        Wsk = perm.tile([128, K_DIM, NG, C_CLS, D_DIM], bf16)  # [r,(k,g,c,d)]
        if FP8_T:
            WT = perm.tile([128, K_DIM, 2, 2 * I_CAPS], f8)
        else:
            WT = perm.tile([128, K_DIM, 2 * I_CAPS], bf16)
        uTk = perm.tile([128, K_DIM, NG, B_PER], bf16)      # u[b, 128g+r, k]
        # exp(L); layout [r, p, ch, g, b] so per-(p,ch) slices are contiguous
        cEa = perm.tile([128, 5, 2, NG, B_PER], bf16, name="cEa")
        cEb = perm.tile([128, 5, 2, NG, B_PER], bf16, name="cEb")
        recT = perm.tile([128, NG, B_PER], bf16, name="recTt")    # 1/den i-major
        if FP8_T:
            # DoubleRow rhs: [32p+8cc+d//2, d%2, 64cc+b]; pass 4 in vT4 rows 0:16
            vT = perm.tile([128, 2, 128], f8)
            vT4 = perm.tile([128, 2, 128], f8)
        else:
            vT = perm.tile([128, 128], bf16)         # block-diag v^T classes 0-7
            vT4 = perm.tile([128, 128], bf16)        # rows 0:32: classes 8,9
        v_sb = perm.tile([64, C_CLS, D_DIM], f32, name="vsbt")
        s_sb = perm.tile([64, C_CLS, D_DIM], f32, name="ssbt")
        eyeb_sb = perm.tile([128, 128], bf16)
        eyef_sb = perm.tile([64, 64], f32, name="eyef32t")
        den = perm.tile([128, NG, B_PER], bf16, name="dent")
        dtmp = perm.tile([128, NG, B_PER], bf16, name="dtmpt")

        # PSUM budget (8 banks): pt2 = 5, lb = 2, sh = 1
        psT = tc.alloc_tile_pool(name="psT", bufs=1, space="PSUM")
        psL = tc.alloc_tile_pool(name="psL", bufs=2, space="PSUM")
        psS = tc.alloc_tile_pool(name="psS", bufs=1, space="PSUM")

        # ---------------- input DMA: s0 feeds first, WT row-chunks last ---
        for k0 in range(0, K_DIM, 2):
            nc.sync.dma_start(uTk[:, k0 : k0 + 2], uTk_in[:, k0 : k0 + 2])
            nc.sync.dma_start(Wsk[:, k0 : k0 + 2], wsk_in[:, k0 : k0 + 2])
        nc.sync.dma_start(eyeb_sb[:], eyebf[:])
        nc.sync.dma_start(eyef_sb[:], eyef32[:])
        if FP8_T:
            for p5 in range(4):
                nc.sync.dma_start(
                    WT[32 * p5 : 32 * p5 + 16, :, :, 0:I_CAPS],
                    wt_in[32 * p5 : 32 * p5 + 16, :, :, 0:I_CAPS],
                )
            nc.sync.dma_start(WT[0:16, :, :, I_CAPS:], wt_in[0:16, :, :, I_CAPS:])
        else:
            for rr in range(0, 128, 32):
                nc.sync.dma_start(
                    WT[rr : rr + 32, :, 0:I_CAPS],
                    wt_in[rr : rr + 32, :, 0:I_CAPS],
                )
            nc.sync.dma_start(WT[0:32, :, I_CAPS:], wt_in[0:32, :, I_CAPS:])

        nc.gpsimd.memset(vT[:], 0.0)
        nc.gpsimd.memset(vT4[:], 0.0)

        # PE warmup filler: keeps the tensor engine busy/ramped through the
        # input-DMA window.
        wu = perm.tile([128, 512], bf16, name="wut")
        nc.vector.memset(wu[:], 0.0)

        itp = tc.alloc_tile_pool(name="itp", bufs=2)
        smp = tc.alloc_tile_pool(name="smp", bufs=3)

        pt2 = psT.tile([128, 2, I_CAPS], f32, name="pt2")  # manual dbl-buffer
        sh = psS.tile([128, 512], f32, name="shps")  # carved small psum bank

        def pe_warmup(n):
            for _ in range(n):
                nc.tensor.matmul(
                    pt2[:, 0, 0:512], eyeb_sb[:], wu[:], start=True, stop=True
                )

        # ---------------- squash (single-act-table variant) ---------------
        # fac = sqrt(n2)/(1+n2) (the +EPS guard is negligible: |s| >> eps);
        # sqrt via exp(0.5*ln) keeps ACT on the {Copy,Exp,Ln} table.
        def squash_pair(p):
            sv = s_sb[:, 2 * p : 2 * p + 2, :]
            sqd = smp.tile([64, D_DIM], f32, tag="sqd", bufs=2)
            n2p = smp.tile([64, 2], f32, tag="n2p", bufs=2)
            lnp = smp.tile([64, 2], f32, tag="lnp", bufs=2)
            nrp = smp.tile([64, 2], f32, tag="nrp", bufs=2)
            dnp = smp.tile([64, 2], f32, tag="dnp", bufs=2)
            rcp = smp.tile([64, 2], f32, tag="rcp", bufs=2)
            fcp = smp.tile([64, 2], f32, tag="fcp", bufs=2)
            for cc in range(2):
                nc.vector.tensor_tensor_reduce(
                    sqd[:], sv[:, cc], sv[:, cc], 1.0, 0.0,
                    OP.mult, OP.add, n2p[:, cc : cc + 1],
                )
            nc.scalar.activation(lnp[:], n2p[:], AF.Ln)
            nc.scalar.activation(nrp[:], lnp[:], AF.Exp, scale=0.5)  # sqrt(n2)
            nc.vector.tensor_scalar_add(dnp[:], n2p[:], 1.0)
            nc.vector.reciprocal(rcp[:], dnp[:])
            nc.vector.tensor_mul(fcp[:], nrp[:], rcp[:])
            for cc in range(2):
                c = 2 * p + cc
                nc.vector.tensor_scalar_mul(
                    v_sb[:, c, :], sv[:, cc, :], fcp[:, cc : cc + 1]
                )

        # ---------------- vT staging: PE transpose + DVE copies -----------
        def vT_stage_pair(p):
            """v_sb pair -> block-diagonal vT slots, fully on-chip.
            bf16: vT[32p+16cc+d, 64cc+b] = v[b, 2p+cc, d]
            fp8 (DoubleRow): vT[32p+8cc+d//2, d%2, 64cc+b]."""
            r0 = 32 * p if p < 4 else 0
            dst = vT if p < 4 else vT4
            idn = eyef_sb[:]
            tps = sh[:, 192:256]  # [128, 64] f32 transpose staging
            with nc.allow_low_precision(reason="vT staging dtype narrowing"):
                if FP8_T:
                    for ko in range(2):
                        src_ = v_sb[:, 2 * p : 2 * p + 2, ko::2].rearrange(
                            "b c d -> b (c d)"
                        )
                        nc.tensor.transpose(
                            tps[r0 : r0 + 16, :], src_, idn,
                            tile_position=(0, r0),
                        )
                        for cc in range(2):
                            nc.vector.tensor_copy(
                                dst[r0 + 8 * cc : r0 + 8 * cc + 8, ko,
                                    64 * cc : 64 * cc + 64],
                                tps[r0 + 8 * cc : r0 + 8 * cc + 8, :],
                            )
                else:
                    src_ = v_sb[:, 2 * p : 2 * p + 2, :].rearrange(
                        "b c d -> b (c d)"
                    )
                    nc.tensor.transpose(
                        tps[r0 : r0 + 32, :], src_, idn, tile_position=(0, r0)
                    )
                    for cc in range(2):
                        nc.vector.tensor_copy(
                            dst[r0 + 16 * cc : r0 + 16 * cc + 16,
                                64 * cc : 64 * cc + 64],
                            tps[r0 + 16 * cc : r0 + 16 * cc + 16, :],
                        )

        # ---------------- s0: chunked accumulation against the DMA stream -
        def s0_phase():
            s0ps = sh[0:64, 0:160].rearrange(
                "b (p c d) -> b p c d", p=5, c=2
            )
            for kc in range(4):
                pe_warmup(WU_C)
                for pp in range(5):
                    view = s0ps[:, pp].rearrange("b c d -> b (c d)")
                    for k in (2 * kc, 2 * kc + 1):
                        for g in range(NG):
                            nc.tensor.matmul(
                                view,
                                uTk[:, k, g, :],
                                Wsk[:, k, g, 2 * pp : 2 * pp + 2, :].rearrange(
                                    "r c d -> r (c d)"
                                ),
                                start=(kc == 0 and k == 0 and g == 0),
                                stop=(kc == 3 and k == 7 and g == NG - 1),
                            )
            for pp in range(5):
                nc.scalar.activation(
                    s_sb[:, 2 * pp : 2 * pp + 2, :].rearrange(
                        "b c d -> b (c d)"
                    ),
                    s0ps[:, pp].rearrange("b c d -> b (c d)"),
                    AF.Copy,
                    scale=0.1,
                )
                squash_pair(pp)
                vT_stage_pair(pp)

        # ---------------- L round -----------------------------------------
        def T_mms(p, k, pt):
            if p < 4:
                row0, col0 = 32 * p, 0
            else:
                row0, col0 = 0, I_CAPS
            if FP8_T:
                vrhs = (vT if p < 4 else vT4)[row0 : row0 + 16, :, :]
            else:
                vrhs = (vT if p < 4 else vT4)[row0 : row0 + 32, :]
            for g in range(NG):
                if FP8_T:
                    nc.tensor.matmul(
                        pt[:, 128 * g : 128 * (g + 1)],
                        WT[row0 : row0 + 16, k, :,
                           col0 + 128 * g : col0 + 128 * (g + 1)],
                        vrhs,
                        start=True,
                        stop=True,
                        tile_position=(row0, 0),
                        perf_mode=PM.DoubleRow,
                    )
                else:
                    nc.tensor.matmul(
                        pt[:, 128 * g : 128 * (g + 1)],
                        WT[row0 : row0 + 32, k,
                           col0 + 128 * g : col0 + 128 * (g + 1)],
                        vrhs,
                        start=True,
                        stop=True,
                        tile_position=(row0, 0),
                    )

        def consumer(p, k, pt, P):
            ubc = uTk[:, k].rearrange("r g b -> r g () b").to_broadcast(
                (128, NG, 2, B_PER)
            )
            pk = P[:, k].rearrange("r g (c b) -> r g c b", c=2)
            zset = Z4_KS if p == 4 else Z_KS
            if k in zset:
                # fused: P_k = T_k(PSUM) * u_k on DVE
                nc.vector.tensor_tensor(
                    pk,
                    pt[:].rearrange("r (g c b) -> r g c b", g=NG, c=2),
                    ubc,
                    OP.mult,
                )
            else:
                Tp = itp.tile([128, NG, 128], bf16, tag="tp", bufs=6)
                nc.scalar.copy(Tp[:].rearrange("r g q -> r (g q)"), pt[:])
                tv = Tp[:].rearrange("r g (c b) -> r g c b", c=2)
                if k in POOL_MUL_KS:
                    nc.gpsimd.tensor_tensor(pk, tv, ubc, OP.mult)
                else:
                    nc.vector.tensor_tensor(pk, tv, ubc, OP.mult)

        def L_round(j, bts, pre_units=None):
            cE_prev, cE = (None, cEa) if j == 0 else (cEa, cEb)
            lpbs = {}
            Ps = {}

            def eye_block(p, B):
                """All 8 k-matmuls of eye block B for pass p, then its exp
                (and round-2 cE product) immediately — one lb tile at a time
                so two PSUM banks suffice and exp sits early in the ACT FIFO."""
                s, e = BLK[B]
                lb = psL.tile([128, 384], f32, tag="lb", bufs=2,
                              name=f"lb{p}_{B}")
                Pf = Ps[p][:].rearrange("r k g q -> r k (g q)")
                for k in range(K_DIM):
                    nc.tensor.matmul(
                        lb[:, 0 : e - s],
                        eyeb_sb[:],
                        Pf[:, k, s:e],
                        start=(k == 0),
                        stop=(k == K_DIM - 1),
                    )
                g0 = 3 * B
                lpv = lb[:, 0 : e - s].rearrange(
                    "r (g c b) -> r g c b", g=3, c=2, b=B_PER
                )
                cgv = cE[:, p, :, g0 : g0 + 3, :].rearrange(
                    "r c g b -> r g c b"
                )
                if j == 0:
                    nc.scalar.activation(
                        cgv, lpv, AF.Exp,
                        scale=(1.0 / W8SCALE if FP8_T else 1.0),
                    )
                else:
                    Er = itp.tile([128, 3, 2, B_PER], bf16, tag="er", bufs=3)
                    nc.scalar.activation(
                        Er[:], lpv, AF.Exp,
                        scale=(1.0 / W8SCALE if FP8_T else 1.0),
                    )
                    pgv = cE_prev[:, p, :, g0 : g0 + 3, :].rearrange(
                        "r c g b -> r g c b"
                    )
                    ce_eng = nc.gpsimd if (CE_POOL and B < 2) else nc.vector
                    ce_eng.tensor_tensor(cgv, Er[:], pgv, OP.mult)
                if B == 2:
                    # progressive denominator folds once the pass is complete
                    eng = nc.gpsimd if (FOLDS_POOL and p < 3) else nc.vector
                    eng.tensor_tensor(
                        bts[p][:], cE[:, p, 0], cE[:, p, 1], OP.add
                    )
                    if p == 1:
                        eng.tensor_tensor(
                            bts[0][:], bts[0][:], bts[1][:], OP.add
                        )
                    elif p == 3:
                        eng.tensor_tensor(
                            bts[2][:], bts[2][:], bts[3][:], OP.add
                        )
                        nc.vector.tensor_tensor(
                            dtmp[:], bts[0][:], bts[2][:], OP.add
                        )

            def ce_folds(p):
                pass  # folded into eye_block(B=2)

            for p in range(5):
                if pre_units is not None and pre_units[p] is not None:
                    pre_units[p]()
                Ps[p] = itp.tile([128, K_DIM, NG, 128], bf16, tag="pp", name=f"P{p}")
                for k in range(K_DIM):
                    pt = pt2[:, k % 2, :]
                    T_mms(p, k, pt)
                    if p >= 1 and k in (0, 1, 2):
                        eye_block(p - 1, k)
                    consumer(p, k, pt, Ps[p])
                if p >= 1 and j == 1:
                    ce_folds(p - 1)
            for B in range(3):
                eye_block(4, B)
            if j == 1:
                ce_folds(4)

        def softmax_phase(bts):
            nc.vector.tensor_tensor(den[:], dtmp[:], bts[4][:], OP.add)
            with nc.allow_low_precision(reason="softmax reciprocal to bf16 ok"):
                nc.vector.reciprocal(
                    recT[:].rearrange("r g b -> r (g b)"),
                    den[:].rearrange("r g b -> r (g b)"),
                )

        # ---------------- routed s-phase ----------------------------------
        def uTs_mul():
            uTs = itp.tile([128, K_DIM, NG, B_PER], bf16, tag="uts", bufs=1)
            nc.vector.tensor_tensor(
                uTs[:],
                uTk[:],
                recT[:].rearrange("r g b -> r () g b").to_broadcast(
                    (128, K_DIM, NG, B_PER)
                ),
                OP.mult,
            )
            return uTs

        def xc_front(c, cE, uTs, xc_dma, gpool):
            p, ch = c // 2, c % 2
            xc = itp.tile(
                [128, K_DIM, NG, B_PER], bf16,
                tag=("xcd" if c in xc_dma else "xc"),
                bufs=(1 if c in xc_dma else 2),
            )
            cbc = cE[:, p, ch].rearrange("r g b -> r () g b").to_broadcast(
                (128, K_DIM, NG, B_PER)
            )
            if c in xc_dma:
                # xc = uTs (SP DMA copy), then xc *= cE_c (gpsimd DMA with
                # CCE multiply; src broadcast over k)
                nc.sync.dma_start(xc[:], uTs[:])
                nc.gpsimd.dma_start(xc[:], cbc, accum_op=OP.mult)
            elif c in XC_POOL:
                nc.gpsimd.tensor_tensor(xc[:], uTs[:], cbc, OP.mult)
            elif gpool > 0:
                gs = NG - gpool
                cb = cE[:, p, ch].rearrange("r g b -> r () g b")
                nc.vector.tensor_tensor(
                    xc[:, :, 0:gs],
                    uTs[:, :, 0:gs],
                    cb[:, :, 0:gs].to_broadcast((128, K_DIM, gs, B_PER)),
                    OP.mult,
                )
                nc.gpsimd.tensor_tensor(
                    xc[:, :, gs:],
                    uTs[:, :, gs:],
                    cb[:, :, gs:].to_broadcast((128, K_DIM, gpool, B_PER)),
                    OP.mult,
                )
            else:
                nc.vector.tensor_tensor(xc[:], uTs[:], cbc, OP.mult)
            return xc

        def s_mm(c, xc):
            o = 160 + 16 * (c % 2)
            ps = sh[0:64, o : o + D_DIM]
            n = 0
            for k in range(K_DIM):
                for g in range(NG):
                    nc.tensor.matmul(
                        ps,
                        xc[:, k, g, :],
                        Wsk[:, k, g, c, :],
                        start=(n == 0),
                        stop=(n == K_DIM * NG - 1),
                    )
                    n += 1
            nc.scalar.copy(s_sb[:, c, :], ps)

        def s_work(c, cE, uTs, xc_dma, gpool):
            xc = xc_front(c, cE, uTs, xc_dma, gpool)
            s_mm(c, xc)

        def out_pair(p):
            nc.sync.dma_start(
                v_out[:, 2 * p : 2 * p + 2, :], v_sb[:, 2 * p : 2 * p + 2, :]
            )

        # ---------------- main flow ----------------------------------------
        s0_phase()
        pe_warmup(WU_T)

        def mkbts(j):
            return [
                smp.tile(
                    [128, NG, B_PER], bf16, tag=f"sm{i}", bufs=2,
                    name=f"bt{i}_{j}",
                )
                for i in range(5)
            ]

        bts0 = mkbts(0)
        L_round(0, bts0)
        softmax_phase(bts0)

        # s-phase(0) interleaved with L-round(1): round-2 T matmuls for pass
        # p follow immediately after vT pair p is staged.
        uTs0 = uTs_mul()

        def sw0(c):
            s_work(c, cEa, uTs0, XC_DMA0, XC_GPOOL)

        for c in (0, 1, 2, 3):
            sw0(c)
        squash_pair(0)

        def pre(pair_s, sq_p, st_p):
            def f():
                if pair_s is not None:
                    sw0(2 * pair_s)
                    sw0(2 * pair_s + 1)
                if sq_p is not None:
                    squash_pair(sq_p)
                if st_p is not None:
                    vT_stage_pair(st_p)
            return f

        bts1 = mkbts(1)
        L_round(
            1,
            bts1,
            pre_units=[
                pre(2, 1, 0),
                pre(3, 2, 1),
                pre(4, 3, 2),
                pre(None, 4, 3),
                pre(None, None, 4),
            ],
        )
        softmax_phase(bts1)

        # final s-phase
        uTs1 = uTs_mul()
        for c in range(C_CLS):
            s_work(c, cEb, uTs1, XC_DMA1, XC_GPOOL1)
            if c % 2 == 1:
                squash_pair(c // 2)
                out_pair(c // 2)

        for pool in (smp, itp, psS, psL, psT, perm):
            try:
                pool.release()
            except Exception:
                pass

    nc.compile()
    return nc


def _consts():
    import ml_dtypes

    return {
        "eyebf": np.eye(128, dtype=np.float32).astype(ml_dtypes.bfloat16),
        "eyef32": np.eye(64, dtype=np.float32),
    }


def _prep_w(W0):
    """Host-side layout marshalling of the replicated weights (pure
    permutation + bf16 cast; done once, shared by all cores)."""
    import ml_dtypes

    bf = ml_dtypes.bfloat16
    W0 = np.ascontiguousarray(W0, dtype=np.float32)  # [1152, 10, 16, 8]
    wsk = np.ascontiguousarray(
        W0.reshape(NG, 128, C_CLS, D_DIM, K_DIM).transpose(1, 4, 0, 2, 3)
    ).astype(bf)  # [128, k, g, c, d]
    if FP8_T:
        f8 = ml_dtypes.float8_e4m3
        wt = np.zeros((128, K_DIM, 2, 2 * I_CAPS), dtype=f8)
        Ws = (W0 * W8SCALE).astype(f8)
        for p in range(5):
            r0, c0 = (32 * p, 0) if p < 4 else (0, I_CAPS)
            for cc in range(2):
                for d in range(D_DIM):
                    # [i, k] -> row r0+8cc+d//2, ko=d%2
                    wt[r0 + 8 * cc + d // 2, :, d % 2, c0 : c0 + I_CAPS] = Ws[
                        :, 2 * p + cc, d, :
                    ].T
        return wsk, wt
    wt = np.zeros((128, K_DIM, 2 * I_CAPS), dtype=bf)
    wt[:, :, 0:I_CAPS] = (
        W0[:, 0:8].transpose(1, 2, 3, 0).reshape(128, K_DIM, I_CAPS).astype(bf)
    )  # rows 16c+d, classes 0-7
    wt[0:32, :, I_CAPS:] = (
        W0[:, 8:10].transpose(1, 2, 3, 0).reshape(32, K_DIM, I_CAPS).astype(bf)
    )  # rows 16(c-8)+d, classes 8,9
    return wsk, wt


def _prep_u(ush):
    import ml_dtypes

    return np.ascontiguousarray(
        ush.reshape(B_PER, NG, 128, K_DIM).transpose(2, 3, 1, 0)
    ).astype(ml_dtypes.bfloat16)  # [128, k, g, b]


def get_nc():
    if "nc" not in _CACHE:
        _CACHE["nc"] = _build()
    return _CACHE["nc"]


def make_in_maps(u, W):
    consts = _consts()
    wsk, wt = _prep_w(W[0])
    in_maps = []
    for core in range(N_CORES):
        sh = np.ascontiguousarray(
            u[core * B_PER : (core + 1) * B_PER], dtype=np.float32
        )
        in_maps.append(
            {
                "uTk_h": _prep_u(sh),
                "wsk_h": wsk,
                "wt_h": wt,
                **consts,
            }
        )
    return in_maps


def kernel(u: np.ndarray, W: np.ndarray) -> np.ndarray:
    from concourse.bass_utils import run_bass_kernel_spmd

    nc = get_nc()
    in_maps = make_in_maps(u, W)
    res = run_bass_kernel_spmd(nc, in_maps, list(range(N_CORES)))
    out = np.concatenate([res.results[i]["v"] for i in range(N_CORES)], axis=0)
    return out.astype(np.float32)


# revision 42
# speedup vs baseline: 1.4452x; 1.4452x over previous
"""DigitCaps (capsule routing) Trainium2 Bass kernel, v2.

u [512, 1152, 8] f32, W [1, 1152, 10, 16, 8] f32 -> v [512, 10, 16] f32
(3 dynamic-routing iterations, softmax over 10 classes).

Pure data-parallel: batch 64 per core x 8 cores; everything on-chip;
u_hat (377MB) never materialized. Per routing iteration:
  T[b,i,c,k] = sum_d W[i,c,d,k] v[b,c,d]     PE -> PSUM
  evac to bf16 (ACT) or fused mul (DVE-from-PSUM), P = T*u
  Linc[b,i,c] = sum_k P                      PE eye-matmul accumulate
  cE = exp(Linc) [* cE_prev]                 ACT exp from PSUM (+DVE mul)
  den folds on GPSIMD; recip DVE
  xc_c = (u*recT) * cE_c                     DVE / GPSIMD split
  s[b,c,:] = sum_{ik} W xc_c                 PE accumulating matmuls
  v = squash(s)
exp(L1+L2) == exp(L1)*exp(L2), so logits are never materialized.

Layouts (per core, B=64):
  i: block g = i//128 (9 blocks), partition r = i%128
  class c = 2p+ch, pass p in [0,5), parity ch in {0,1}
  exp/cE: [r, p, (g, ch, b)]
"""

import os
import numpy as np

N_CORES = 8
B_PER = 64
I_CAPS = 1152
K_DIM = 8
C_CLS = 10
D_DIM = 16
NG = I_CAPS // 128  # 9
EPS = 1e-8

# --- schedule knobs (cost-model balancing) ---
Z_KS = tuple(
    int(x) for x in os.environ.get("KV2_ZKS", "1,3,5,7").split(",") if x != ""
)  # k-indices whose T*u mul reads PSUM directly on DVE (no ACT evac)
POOL_MUL_KS = tuple(
    int(x) for x in os.environ.get("KV2_PMKS", "").split(",") if x != ""
)  # k-indices whose (evac'd) mul runs on GPSIMD
XC_POOL = tuple(
    int(x) for x in os.environ.get("KV2_XCPOOL", "").split(",") if x != ""
)  # classes whose xc mul runs on GPSIMD
XC_DMA0 = tuple(
    int(x) for x in os.environ.get("KV2_XCDMA0", "").split(",") if x != ""
)  # round-1 classes whose xc mul runs as SP-copy + gpsimd DMA-accum-mult
XC_DMA1 = tuple(
    int(x) for x in os.environ.get("KV2_XCDMA1", "").split(",") if x != ""
)  # round-2 classes on the DMA-mult route
XC_GPOOL = int(os.environ.get("KV2_XCGPOOL", "2"))  # trailing g-blocks on Pool
FOLDS_POOL = os.environ.get("KV2_FOLDSPOOL", "1") == "1"
CE_POOL = os.environ.get("KV2_CEPOOL", "1") == "1"
CE_ALT = os.environ.get("KV2_CEALT", "1") == "1"
FP8_T = os.environ.get("KV2_FP8", "0") == "1"  # DoubleRow fp8 T matmuls
W8SCALE = 256.0  # exact power-of-two prescale lifting fp8 W out of subnormals

_CACHE = {}


def _build():
    import concourse.bass as bass
    import concourse.mybir as mybir
    from concourse import tile, bacc

    f32 = mybir.dt.float32
    bf16 = mybir.dt.bfloat16
    f8 = mybir.dt.float8e4
    AF = mybir.ActivationFunctionType
    OP = mybir.AluOpType
    PM = mybir.MatmulPerfMode

    nc = bacc.Bacc()
    uTk_in = nc.dram_tensor(
        "uTk_h", [128, K_DIM, NG, B_PER], bf16, kind="ExternalInput"
    )
    wsk_in = nc.dram_tensor(
        "wsk_h", [128, K_DIM, NG, C_CLS, D_DIM], bf16, kind="ExternalInput"
    )
    # wt cols 0:1152 = rows 16c+d classes 0-7 (all 128 partitions);
    # cols 1152:2304 = rows 16(c-8)+d classes 8,9 (partitions 0:32)
    wdt = f8 if FP8_T else bf16
    if FP8_T:
        # DoubleRow layout: [32p+8cc+d//2, k, d%2, i(+1152 for classes 8,9)]
        wt_in = nc.dram_tensor(
            "wt_h", [128, K_DIM, 2, 2 * I_CAPS], f8, kind="ExternalInput"
        )
    else:
        wt_in = nc.dram_tensor(
            "wt_h", [128, K_DIM, 2 * I_CAPS], bf16, kind="ExternalInput"
        )
    eyebf = nc.dram_tensor("eyebf", [128, 128], bf16, kind="ExternalInput")
    v_out = nc.dram_tensor("v", [B_PER, C_CLS, D_DIM], f32, kind="ExternalOutput")
    vdr = nc.dram_tensor("vdr", [2, 5, B_PER, 2, D_DIM], wdt, kind="Internal")

    with tile.TileContext(nc) as tc:
        perm = tc.alloc_tile_pool(name="perm", bufs=1)
        Wsk = perm.tile([128, K_DIM, NG, C_CLS, D_DIM], bf16)  # [r,(k,g,c,d)]
        if FP8_T:
            WT = perm.tile([128, K_DIM, 2, 2 * I_CAPS], f8)
        else:
            WT = perm.tile([128, K_DIM, 2 * I_CAPS], bf16)
        uTk = perm.tile([128, K_DIM, NG, B_PER], bf16)      # u[b, 128g+r, k]
        # exp(L); layout [r, p, ch, g, b] so per-(p,ch) slices are contiguous
        cEa = perm.tile([128, 5, 2, NG, B_PER], bf16, name="cEa")
        cEb = perm.tile([128, 5, 2, NG, B_PER], bf16, name="cEb")
        recT = perm.tile([128, NG, B_PER], bf16, name="recTt")    # 1/den i-major
        if FP8_T:
            # DoubleRow rhs: [32p+8cc+d//2, d%2, 64cc+b]; pass 4 in vT4 rows 0:16
            vT = perm.tile([128, 2, 128], f8)
            vT4 = perm.tile([128, 2, 128], f8)
        else:
            vT = perm.tile([128, 128], bf16)         # block-diag v^T classes 0-7
            vT4 = perm.tile([128, 128], bf16)        # rows 0:32: classes 8,9
        v_sb = perm.tile([64, C_CLS, D_DIM], f32, name="vsbt")
        vbf = perm.tile([64, C_CLS, D_DIM], wdt, name="vbft")
        s_sb = perm.tile([64, C_CLS, D_DIM], f32, name="ssbt")
        eyeb_sb = perm.tile([128, 128], bf16)
        den = perm.tile([128, NG, B_PER], bf16, name="dent")
        dtmp = perm.tile([128, NG, B_PER], bf16, name="dtmpt")

        psT = tc.alloc_tile_pool(name="psT", bufs=2, space="PSUM")
        psL = tc.alloc_tile_pool(name="psL", bufs=2, space="PSUM")

        # ---------------- setup: inputs arrive pre-arranged ----
        for k0 in range(0, K_DIM, 2):
            nc.sync.dma_start(uTk[:, k0 : k0 + 2], uTk_in[:, k0 : k0 + 2])
            nc.sync.dma_start(Wsk[:, k0 : k0 + 2], wsk_in[:, k0 : k0 + 2])
        nc.sync.dma_start(eyeb_sb[:], eyebf[:])
        if FP8_T:
            for p5 in range(4):
                nc.sync.dma_start(
                    WT[32 * p5 : 32 * p5 + 16, :, :, 0:I_CAPS],
                    wt_in[32 * p5 : 32 * p5 + 16, :, :, 0:I_CAPS],
                )
            nc.sync.dma_start(WT[0:16, :, :, I_CAPS:], wt_in[0:16, :, :, I_CAPS:])
        else:
            nc.sync.dma_start(WT[:, 0:4, 0:I_CAPS], wt_in[:, 0:4, 0:I_CAPS])
            nc.sync.dma_start(WT[0:32, 0:4, I_CAPS:], wt_in[0:32, 0:4, I_CAPS:])
            nc.sync.dma_start(WT[:, 4:8, 0:I_CAPS], wt_in[:, 4:8, 0:I_CAPS])
            nc.sync.dma_start(WT[0:32, 4:8, I_CAPS:], wt_in[0:32, 4:8, I_CAPS:])

        nc.gpsimd.memset(vT[:], 0.0)
        nc.gpsimd.memset(vT4[:], 0.0)

        # PE warmup: keep the tensor engine continuously busy through the
        # input-DMA window so s0 and round 1 run at the ramped clock.
        wu = perm.tile([128, 512], bf16, name="wut")
        nc.vector.memset(wu[:], 0.0)

        itp = tc.alloc_tile_pool(name="itp", bufs=2)
        smp = tc.alloc_tile_pool(name="smp", bufs=3)

        def pe_warmup(n):
            for _ in range(n):
                wt = psT.tile([128, I_CAPS], f32, tag="pt")
                nc.tensor.matmul(
                    wt[:, 0:512], eyeb_sb[:], wu[:], start=True, stop=True
                )

        def s_phase_s0_pair(pp):
            """Uniform-c s for classes 2pp,2pp+1 only, so the first routing
            round can start on a class pair before s0 fully finishes."""
            pst = psL.tile([128, 512], f32, tag="lp")
            ps = pst[0:64, 0 : 2 * D_DIM]
            n = 0
            for k in range(K_DIM):
                for g in range(NG):
                    nc.tensor.matmul(
                        ps,
                        uTk[:, k, g, :],
                        Wsk[:, k, g, 2 * pp : 2 * pp + 2, :].rearrange(
                            "r c d -> r (c d)"
                        ),
                        start=(n == 0),
                        stop=(n == K_DIM * NG - 1),
                    )
                    n += 1
            nc.scalar.activation(
                s_sb[:, 2 * pp : 2 * pp + 2, :].rearrange("b c d -> b (c d)"),
                ps,
                AF.Copy,
                scale=0.1,
            )

        def squash_pair(p, final=False):
            """squash for classes 2p, 2p+1 only; writes bf16 vbf slices
            (or f32 v_sb when final).
            fac = n2 / ((1 + n2) * (sqrt(n2) + EPS)), v = fac * s."""
            sqp = smp.tile([64, 2, D_DIM], f32, tag="sqp", bufs=2)
            n2p = smp.tile([64, 2], f32, tag="n2p", bufs=2)
            nrp = smp.tile([64, 2], f32, tag="nrp", bufs=2)
            dnp = smp.tile([64, 2], f32, tag="dnp", bufs=2)
            rcp = smp.tile([64, 2], f32, tag="rcp", bufs=2)
            fcp = smp.tile([64, 2], f32, tag="fcp", bufs=2)
            sv = s_sb[:, 2 * p : 2 * p + 2, :]
            nc.scalar.square(sqp[:], sv)
            nc.vector.reduce_sum(n2p[:], sqp[:], axis=mybir.AxisListType.X)
            nc.scalar.sqrt(nrp[:], n2p[:])
            nc.vector.tensor_scalar_add(nrp[:], nrp[:], EPS)
            # dnp = (n2p + 1) * nrp
            nc.vector.scalar_tensor_tensor(
                dnp[:], n2p[:], 1.0, nrp[:], OP.add, OP.mult
            )
            nc.vector.reciprocal(rcp[:], dnp[:])
            nc.vector.tensor_mul(fcp[:], n2p[:], rcp[:])
            if final:
                for cc in range(2):
                    c = 2 * p + cc
                    nc.vector.tensor_scalar_mul(
                        v_sb[:, c, :], sv[:, cc, :], fcp[:, cc : cc + 1]
                    )
            else:
                with nc.allow_low_precision(reason="v to bf16 for T matmuls"):
                    for cc in range(2):
                        c = 2 * p + cc
                        nc.vector.tensor_scalar_mul(
                            vbf[:, c, :], sv[:, cc, :], fcp[:, cc : cc + 1]
                        )

        def vT_write_pair(p, slot):
            """Stage classes 2p,2p+1 of vbf in DRAM (SP-initiated)."""
            nc.sync.dma_start(vdr[slot, p], vbf[:, 2 * p : 2 * p + 2, :])

        def vT_read_pair(p, slot):
            """Read a staged pair back transposed into its block-diagonal vT
            slots. bf16: vT[32p+16cc+d, 64cc+b] = v[b,2p+cc,d]. fp8/DoubleRow:
            vT[32p+8cc+d//2, d%2, 64cc+b]. Reads spread over SP/ACT queues."""
            dst_tile = vT if p < 4 else vT4
            r0 = 32 * p if p < 4 else 0
            if FP8_T:
                engs = (nc.sync, nc.scalar)
                n = 0
                for cc in range(2):
                    for ko in range(2):
                        engs[n % 2].dma_start(
                            dst_tile[r0 + 8 * cc : r0 + 8 * cc + 8, ko,
                                     64 * cc : 64 * cc + 64],
                            vdr[slot, p, :, cc, ko::2].rearrange("b d -> d b"),
                        )
                        n += 1
            else:
                for eng, cc in ((nc.sync, 0), (nc.scalar, 1)):
                    eng.dma_start(
                        dst_tile[r0 + 16 * cc : r0 + 16 * cc + 16,
                                 64 * cc : 64 * cc + 64],
                        vdr[slot, p, :, cc, :].rearrange("b d -> d b"),
                    )

        def vT_dma_pair(p, slot):
            vT_write_pair(p, slot)
            vT_read_pair(p, slot)

        def L_front(j, p):
            """T matmuls + evac/mul for pass p; returns the P tile."""
            if p < 4:
                row0, col0 = 32 * p, 0
            else:
                row0, col0 = 0, I_CAPS
            if FP8_T:
                vrhs = (vT if p < 4 else vT4)[row0 : row0 + 16, :, :]
            else:
                vrhs = (vT if p < 4 else vT4)[row0 : row0 + 32, :]
            P = itp.tile([128, K_DIM, NG, 128], bf16, tag="pp")
            for k in range(K_DIM):
                pt = psT.tile([128, I_CAPS], f32, tag="pt")
                for g in range(NG):
                    if FP8_T:
                        nc.tensor.matmul(
                            pt[:, 128 * g : 128 * (g + 1)],
                            WT[row0 : row0 + 16, k, :,
                               col0 + 128 * g : col0 + 128 * (g + 1)],
                            vrhs,
                            start=True,
                            stop=True,
                            tile_position=(row0, 0),
                            perf_mode=PM.DoubleRow,
                        )
                    else:
                        nc.tensor.matmul(
                            pt[:, 128 * g : 128 * (g + 1)],
                            WT[row0 : row0 + 32, k,
                               col0 + 128 * g : col0 + 128 * (g + 1)],
                            vrhs,
                            start=True,
                            stop=True,
                            tile_position=(row0, 0),
                        )
                ubc = uTk[:, k].rearrange("r g b -> r g () b").to_broadcast(
                    (128, NG, 2, B_PER)
                )
                pk = P[:, k].rearrange("r g (c b) -> r g c b", c=2)
                if k in Z_KS:
                    # fused: P_k = T_k(PSUM) * u_k on DVE
                    nc.vector.tensor_tensor(
                        pk,
                        pt[:].rearrange("r (g c b) -> r g c b", g=NG, c=2),
                        ubc,
                        OP.mult,
                    )
                else:
                    Tp = itp.tile([128, NG, 128], bf16, tag="tp", bufs=6)
                    nc.scalar.copy(Tp[:].rearrange("r g q -> r (g q)"), pt[:])
                    tv = Tp[:].rearrange("r g (c b) -> r g c b", c=2)
                    # scalar_tensor_tensor (TensorScalarPtr) runs in the DVE
                    # 4x_2p perf mode on all-SBUF packed bf16 operands; plain
                    # tensor_tensor only reaches 2x.
                    if k in POOL_MUL_KS:
                        nc.gpsimd.tensor_tensor(pk, tv, ubc, OP.mult)
                    else:
                        nc.vector.tensor_tensor(pk, tv, ubc, OP.mult)
            return P

        def L_back(j, p, P, bts, cE_prev, cE):
            """ksum + exp + cE product + denominator folds for pass p."""
            Pf = P[:].rearrange("r k g q -> r k (g q)")

            def cegv(t, g0, g1):
                # [r, g, ch, b] view over g block (enumeration order of Linc)
                return t[:, p, :, g0:g1, :].rearrange("r c g b -> r g c b")

            for g0, g1 in ((0, 4), (4, 8), (8, NG)):
                s, e = 128 * g0, 128 * g1
                lp = psL.tile([128, 512], f32, tag="lp")
                for k in range(K_DIM):
                    nc.tensor.matmul(
                        lp[:, 0 : e - s],
                        eyeb_sb[:],
                        Pf[:, k, s:e],
                        start=(k == 0),
                        stop=(k == K_DIM - 1),
                    )
                lpv = lp[:, 0 : e - s].rearrange(
                    "r (g c b) -> r g c b", c=2, b=B_PER
                )
                if j == 0:
                    nc.scalar.activation(
                        cegv(cE, g0, g1), lpv, AF.Exp,
                        scale=(1.0 / W8SCALE if FP8_T else 1.0),
                    )
                else:
                    Er = itp.tile([128, 512], bf16, tag="er", bufs=3)
                    erv = Er[:, 0 : e - s].rearrange(
                        "r (g c b) -> r g c b", c=2, b=B_PER
                    )
                    nc.scalar.activation(
                        erv, lpv, AF.Exp,
                        scale=(1.0 / W8SCALE if FP8_T else 1.0),
                    )
                    ce_eng = (
                        nc.gpsimd
                        if (CE_POOL and (g0 < 8 or not CE_ALT))
                        else nc.vector
                    )
                    ce_eng.tensor_tensor(
                        cegv(cE, g0, g1), erv, cegv(cE_prev, g0, g1), OP.mult
                    )
            # progressive denominator folds; the last pass's fold and the
            # final partial sum sit on the recT critical chain, so they go
            # to DVE even when the earlier folds run on GPSIMD
            eng = nc.gpsimd if (FOLDS_POOL and p < 3) else nc.vector
            eng.tensor_tensor(bts[p][:], cE[:, p, 0], cE[:, p, 1], OP.add)
            if p == 1:
                eng.tensor_tensor(bts[0][:], bts[0][:], bts[1][:], OP.add)
            elif p == 3:
                eng.tensor_tensor(bts[2][:], bts[2][:], bts[3][:], OP.add)
                # dtmp = (b0+b1) + (b2+b3) ready before pass 4 lands
                nc.vector.tensor_tensor(dtmp[:], bts[0][:], bts[2][:], OP.add)

        def L_phase(j, bts):
            """Software-pipelined: pass p's back-end is emitted after pass
            p+1's front-end so its cross-engine waits are already satisfied
            when they reach the strict-FIFO engine queues."""
            cE_prev, cE = (None, cEa) if j == 0 else (cEa, cEb)
            Ps = [None] * 5
            for p in range(5):
                Ps[p] = L_front(j, p)
                if p >= 1:
                    L_back(j, p - 1, Ps[p - 1], bts, cE_prev, cE)
            L_back(j, 4, Ps[4], bts, cE_prev, cE)
            return cE

        def softmax_phase(bts):
            nc.vector.tensor_tensor(den[:], dtmp[:], bts[4][:], OP.add)
            with nc.allow_low_precision(reason="softmax reciprocal to bf16 ok"):
                nc.vector.reciprocal(
                    recT[:].rearrange("r g b -> r (g b)"),
                    den[:].rearrange("r g b -> r (g b)"),
                )

        def s_phase_routed(cE, pipe_vT, xc_dma):
            uTs = itp.tile([128, K_DIM, NG, B_PER], bf16, tag="uts", bufs=1)
            nc.vector.tensor_tensor(
                uTs[:],
                uTk[:],
                recT[:].rearrange("r g b -> r () g b").to_broadcast(
                    (128, K_DIM, NG, B_PER)
                ),
                OP.mult,
            )
            def xc_front(c):
                p, ch = c // 2, c % 2
                xc = itp.tile(
                    [128, K_DIM, NG, B_PER], bf16,
                    tag=("xcd" if c in xc_dma else "xc"),
                    bufs=(1 if c in xc_dma else 2),
                )
                cbc = cE[:, p, ch].rearrange("r g b -> r () g b").to_broadcast(
                    (128, K_DIM, NG, B_PER)
                )
                if c in xc_dma:
                    # xc = us (SP DMA copy), then xc *= cE_c (gpsimd DMA
                    # with CCE multiply; src broadcast over k)
                    nc.sync.dma_start(xc[:], uTs[:])
                    nc.gpsimd.dma_start(xc[:], cbc, accum_op=OP.mult)
                elif c in XC_POOL:
                    nc.gpsimd.tensor_tensor(xc[:], uTs[:], cbc, OP.mult)
                elif XC_GPOOL > 0:
                    gs = NG - XC_GPOOL
                    cb = cE[:, p, ch].rearrange("r g b -> r () g b")
                    nc.vector.tensor_tensor(
                        xc[:, :, 0:gs],
                        uTs[:, :, 0:gs],
                        cb[:, :, 0:gs].to_broadcast((128, K_DIM, gs, B_PER)),
                        OP.mult,
                    )
                    nc.gpsimd.tensor_tensor(
                        xc[:, :, gs:],
                        uTs[:, :, gs:],
                        cb[:, :, gs:].to_broadcast(
                            (128, K_DIM, XC_GPOOL, B_PER)
                        ),
                        OP.mult,
                    )
                else:
                    nc.vector.tensor_tensor(xc[:], uTs[:], cbc, OP.mult)
                return xc

            def s_back(c, xc):
                pst = psL.tile([128, 512], f32, tag="lp")
                ps = pst[0:64, 0:D_DIM]
                n = 0
                for k in range(K_DIM):
                    for g in range(NG):
                        nc.tensor.matmul(
                            ps,
                            xc[:, k, g, :],
                            Wsk[:, k, g, c, :],
                            start=(n == 0),
                            stop=(n == K_DIM * NG - 1),
                        )
                        n += 1
                nc.scalar.copy(s_sb[:, c, :], ps)

            def pair_done(cdone):
                """Classes 2p,2p+1 are in s_sb: squash the pair; round 1
                DMAs the block into vT so the next round's T matmuls can
                start, round 2 DMAs the f32 result out early."""
                if cdone % 2 != 1:
                    return
                p = cdone // 2
                if pipe_vT:
                    squash_pair(p)
                    vT_dma_pair(p, 1)
                else:
                    squash_pair(p, final=True)
                    nc.sync.dma_start(
                        v_out[:, 2 * p : 2 * p + 2, :], v_sb[:, 2 * p : 2 * p + 2, :]
                    )

            xcs = [None] * C_CLS
            for c in range(C_CLS):
                xcs[c] = xc_front(c)
                if c >= 1:
                    s_back(c - 1, xcs[c - 1])
                    pair_done(c - 1)
            s_back(C_CLS - 1, xcs[C_CLS - 1])
            pair_done(C_CLS - 1)

        # ---------------- main flow ----------------
        kstage = int(os.environ.get("KSTAGE", "99"))
        wu_n = int(os.environ.get("KV2_WU", "8"))
        for p in range(5):
            s_phase_s0_pair(p)
            if p == 0 and wu_n:
                # low-priority gap fillers: run only when the PE would
                # otherwise idle waiting on the input DMA stream
                pe_warmup(wu_n)
            squash_pair(p)
            vT_dma_pair(p, 0)
        if kstage >= 1:
            for j in range(2):
                bts = []
                for i in range(5):
                    bti = smp.tile(
                        [128, NG, B_PER], bf16, tag=f"sm{i}", bufs=2,
                        name=f"bt{i}_{j}",
                    )
                    bts.append(bti)
                cE = L_phase(j, bts)
                if kstage == 1 + 3 * j:
                    break
                softmax_phase(bts)
                if kstage == 2 + 3 * j:
                    break
                s_phase_routed(
                    cE, pipe_vT=(j == 0), xc_dma=(XC_DMA0 if j == 0 else XC_DMA1)
                )
                if kstage == 3 + 3 * j:
                    break

        for pool in (smp, itp, psL, psT, perm):
            try:
                pool.release()
            except Exception:
                pass

    nc.compile()
    return nc


def _consts():
    import ml_dtypes

    return {"eyebf": np.eye(128, dtype=np.float32).astype(ml_dtypes.bfloat16)}


def _prep_w(W0):
    """Host-side layout marshalling of the replicated weights (pure
    permutation + bf16 cast; done once, shared by all cores)."""
    import ml_dtypes

    bf = ml_dtypes.bfloat16
    W0 = np.ascontiguousarray(W0, dtype=np.float32)  # [1152, 10, 16, 8]
    wsk = np.ascontiguousarray(
        W0.reshape(NG, 128, C_CLS, D_DIM, K_DIM).transpose(1, 4, 0, 2, 3)
    ).astype(bf)  # [128, k, g, c, d]
    if FP8_T:
        f8 = ml_dtypes.float8_e4m3
        wt = np.zeros((128, K_DIM, 2, 2 * I_CAPS), dtype=f8)
        Ws = (W0 * W8SCALE).astype(f8)
        for p in range(5):
            r0, c0 = (32 * p, 0) if p < 4 else (0, I_CAPS)
            for cc in range(2):
                for d in range(D_DIM):
                    # [i, k] -> row r0+8cc+d//2, ko=d%2
                    wt[r0 + 8 * cc + d // 2, :, d % 2, c0 : c0 + I_CAPS] = Ws[
                        :, 2 * p + cc, d, :
                    ].T
        return wsk, wt
    wt = np.zeros((128, K_DIM, 2 * I_CAPS), dtype=bf)
    wt[:, :, 0:I_CAPS] = (
        W0[:, 0:8].transpose(1, 2, 3, 0).reshape(128, K_DIM, I_CAPS).astype(bf)
    )  # rows 16c+d, classes 0-7
    wt[0:32, :, I_CAPS:] = (
        W0[:, 8:10].transpose(1, 2, 3, 0).reshape(32, K_DIM, I_CAPS).astype(bf)
    )  # rows 16(c-8)+d, classes 8,9
    return wsk, wt


def _prep_u(ush):
    import ml_dtypes

    return np.ascontiguousarray(
        ush.reshape(B_PER, NG, 128, K_DIM).transpose(2, 3, 1, 0)
    ).astype(ml_dtypes.bfloat16)  # [128, k, g, b]


def get_nc():
    if "nc" not in _CACHE:
        _CACHE["nc"] = _build()
    return _CACHE["nc"]


def make_in_maps(u, W):
    consts = _consts()
    wsk, wt = _prep_w(W[0])
    in_maps = []
    for core in range(N_CORES):
        sh = np.ascontiguousarray(
            u[core * B_PER : (core + 1) * B_PER], dtype=np.float32
        )
        in_maps.append(
            {
                "uTk_h": _prep_u(sh),
                "wsk_h": wsk,
                "wt_h": wt,
                **consts,
            }
        )
    return in_maps


def kernel(u: np.ndarray, W: np.ndarray) -> np.ndarray:
    from concourse.bass_utils import run_bass_kernel_spmd

    nc = get_nc()
    in_maps = make_in_maps(u, W)
    res = run_bass_kernel_spmd(nc, in_maps, list(range(N_CORES)))
    out = np.concatenate([res.results[i]["v"] for i in range(N_CORES)], axis=0)
    return out.astype(np.float32)

